# revision 1
# baseline (speedup 1.0000x reference)
"""Bilateral denoising/sharpening filter on 8 trn2 NeuronCores (data parallel,
2 images per core; host reflect-pads and cuts each image into 36x36 halo'd
patches, one patch per SBUF partition, so every filter tap is a free-dim view).

Pair-symmetric formulation: w(p,q) = w(q,p), so each unordered neighbor pair
is computed once (12 pairs instead of 24 taps) on an extended (<=34x34)
domain, then contributes to num/den twice: once at p (gather) and once at q
(scatter).  Both contributions are TensorEngine identity-matmul accumulations
into fp32 PSUM using shifted SBUF views.  Color distance uses a custom fused
(a-b)^2 DVE op on fp32 inputs; channel sums / exp output / products run in
fp16 (2x DVE mode).  The dominant center tap stays exact fp32.
"""

import sys

sys.path.insert(0, "/opt/trn_rl_repo")

import numpy as np

KERNEL_SIZE = 5
SIGMA_S = 1.0
SIGMA_R = 0.04
INV2SR2 = 0.5 / (SIGMA_R * SIGMA_R)

B, H, W, C = 16, 512, 512, 3
NCORES = 8
IMGS_PER_CORE = B // NCORES
PATCH = 32
HALO = 36
NPS = H // PATCH
PATCHES_PER_CORE = IMGS_PER_CORE * NPS * NPS
ROUNDS = PATCHES_PER_CORE // 128

_CACHE = {}

PAIRS = [
    (dy, dx)
    for dy in range(KERNEL_SIZE)
    for dx in range(KERNEL_SIZE)
    if (dy < 2) or (dy == 2 and dx < 2)
]


def _space_kernel():
    x = np.arange(KERNEL_SIZE, dtype=np.float32) - (KERNEL_SIZE // 2)
    g = np.exp(-(x * x) / np.float32(2.0 * SIGMA_S * SIGMA_S)).astype(np.float32)
    g = (g / g.sum()).astype(np.float32)
    return np.outer(g, g).astype(np.float32)


def _register_sqdiff():
    import concourse.dve_ops as dve_ops
    from concourse.dve_spec import Spec, Src0, Src1, sq, lower
    from concourse.dve_uop import DveOpSpec

    name = "SQDIFF_BILAT"
    if name in dve_ops._SUB_OPCODE_FOR_NAME:
        return next(o for o in dve_ops.OPS if o.name == name)
    spec = Spec(
        body=sq(Src0 - Src1),
        reference=lambda in0, in1, s0, s1, imm2: (
            (in0.astype(np.float32) - in1.astype(np.float32)) ** 2
        ).astype(np.float32),
    )
    opcode = dve_ops._CUSTOM_DVE_ROW_BASE + len(dve_ops.OPS)
    shas = {}
    for ver in ("v3", "v4"):
        u = lower(spec, ver=ver)
        shas[ver] = DveOpSpec(name=name, opcode=opcode, uops=u, rd1_en=True).sha(ver)
    op = dve_ops.DveOp(name, spec, subdim=False, uops_sha=shas)
    dve_ops.OPS.append(op)
    dve_ops.CUSTOM_DVE_SPECS[name] = spec
    dve_ops._SUB_OPCODE_FOR_NAME[name] = opcode
    return op


def _build_module(repeat=1):
    import concourse.bacc as bacc
    import concourse.mybir as mybir
    import concourse.tile as tile

    SQDIFF = _register_sqdiff()
    f32 = mybir.dt.float32
    bf16 = mybir.dt.float16  # fp16: same 2x DVE modes, 3 more mantissa bits
    A = mybir.AluOpType
    sk = _space_kernel()
    sk22 = float(sk[2, 2])

    nc = bacc.Bacc("TRN2", target_bir_lowering=False, debug=False)
    xpat = nc.dram_tensor("xpat", [ROUNDS, 128, C, HALO, HALO], f32, kind="ExternalInput")
    identb = nc.dram_tensor("identb", [128, 128], bf16, kind="ExternalInput")  # fp16
    identsk = nc.dram_tensor("identsk", [128, 128], f32, kind="ExternalInput")
    lnsk = nc.dram_tensor("lnsk", [128, 32], f32, kind="ExternalInput")
    outd = nc.dram_tensor("out", [ROUNDS, 128, C, PATCH, PATCH], f32, kind="ExternalOutput")

    def rng_ax(d):
        # union of gather [2,34) and scatter [2-d,34-d) index ranges
        if d >= 0:
            return 2 - d, 34
        return 2, 34 - d

    with tile.TileContext(nc) as tc:
        with (
            tc.tile_pool(name="const", bufs=1) as cpool,
            tc.tile_pool(name="xin", bufs=2) as xpool,
            tc.tile_pool(name="work", bufs=2) as wpool,
            tc.tile_pool(name="outp", bufs=2) as opool,
            tc.tile_pool(name="epi", bufs=1) as epool,
            tc.tile_pool(name="psum", bufs=1, space="PSUM") as ppool,
        ):
            identb_t = cpool.tile([128, 128], bf16, tag="identb")
            nc.sync.dma_start(identb_t[:], identb[:])
            identsk_t = cpool.tile([128, 128], f32, tag="identsk")
            nc.sync.dma_start(identsk_t[:], identsk[:])
            lnsk_t = cpool.tile([128, 32], f32, tag="lnsk")
            nc.sync.dma_start(lnsk_t[:], lnsk[:])

            for r in [rr for _ in range(repeat) for rr in range(ROUNDS)]:
                xt = xpool.tile([128, C, HALO, HALO], f32, tag="xt")
                nc.sync.dma_start(xt[:], xpat[r])
                xbe = xpool.tile([128, C, HALO, HALO], bf16, tag="xbe")
                nc.vector.tensor_copy(xbe[:], xt[:])

                num = [
                    ppool.tile([128, PATCH, PATCH], f32, tag=f"num{c}", name=f"num{c}")
                    for c in range(C)
                ]
                den = ppool.tile([128, PATCH, PATCH], f32, tag="den")

                xc = xt[:, :, 2 : 2 + PATCH, 2 : 2 + PATCH]
                for c in range(C):
                    for hh in range(2):
                        nc.tensor.matmul(
                            num[c][:, 16 * hh : 16 * hh + 16],
                            identsk_t[:],
                            xc[:, c, 16 * hh : 16 * hh + 16],
                            start=True,
                            stop=False,
                        )

                for ti, (dy, dx) in enumerate(PAIRS):
                    d_y, d_x = dy - 2, dx - 2
                    u0y, u1y = rng_ax(d_y)
                    u0x, u1x = rng_ax(d_x)
                    sy, sx = u1y - u0y, u1x - u0x

                    q = wpool.tile([128, C, 34, 34], bf16, tag="q")
                    for c in range(C):
                        nc.vector._custom_dve(
                            SQDIFF,
                            out=q[:, c, :sy, :sx],
                            in0=xt[:, c, u0y:u1y, u0x:u1x],
                            in1=xt[:, c, u0y + d_y : u1y + d_y, u0x + d_x : u1x + d_x],
                        )
                    d2 = wpool.tile([128, 34, 34], bf16, tag="d2")
                    nc.vector.tensor_tensor(
                        d2[:, :sy, :sx], q[:, 0, :sy, :sx], q[:, 1, :sy, :sx], A.add
                    )
                    nc.vector.tensor_tensor(
                        d2[:, :sy, :sx], d2[:, :sy, :sx], q[:, 2, :sy, :sx], A.add
                    )
                    w = wpool.tile([128, 34, 34], bf16, tag="w")
                    nc.scalar.activation(
                        w[:, :sy, :sx],
                        d2[:, :sy, :sx],
                        mybir.ActivationFunctionType.Exp,
                        bias=lnsk_t[:, ti : ti + 1],
                        scale=-float(INV2SR2),
                    )

                    gy, gx = 2 - u0y, 2 - u0x  # gather origin in w tile
                    zy, zx = 2 - d_y - u0y, 2 - d_x - u0x  # scatter origin
                    wg = w[:, gy : gy + 32, gx : gx + 32]
                    ws = w[:, zy : zy + 32, zx : zx + 32]

                    t = wpool.tile([128, C, PATCH, PATCH], bf16, tag="t")
                    u = wpool.tile([128, C, PATCH, PATCH], bf16, tag="u")
                    for c in range(C):
                        nc.vector.tensor_tensor(
                            t[:, c], wg, xbe[:, c, 2 + d_y : 34 + d_y, 2 + d_x : 34 + d_x], A.mult
                        )
                        nc.vector.tensor_tensor(
                            u[:, c], ws, xbe[:, c, 2 - d_y : 34 - d_y, 2 - d_x : 34 - d_x], A.mult
                        )

                    last = ti == len(PAIRS) - 1
                    for c in range(C):
                        for hh in range(2):
                            nc.tensor.matmul(
                                num[c][:, 16 * hh : 16 * hh + 16],
                                identb_t[:],
                                t[:, c, 16 * hh : 16 * hh + 16],
                                start=False,
                                stop=False,
                            )
                            nc.tensor.matmul(
                                num[c][:, 16 * hh : 16 * hh + 16],
                                identb_t[:],
                                u[:, c, 16 * hh : 16 * hh + 16],
                                start=False,
                                stop=last,
                            )
                    for hh in range(2):
                        nc.tensor.matmul(
                            den[:, 16 * hh : 16 * hh + 16],
                            identb_t[:],
                            wg[:, 16 * hh : 16 * hh + 16],
                            start=(ti == 0),
                            stop=False,
                        )
                        nc.tensor.matmul(
                            den[:, 16 * hh : 16 * hh + 16],
                            identb_t[:],
                            ws[:, 16 * hh : 16 * hh + 16],
                            start=False,
                            stop=last,
                        )

                dsb = epool.tile([128, PATCH, PATCH], f32, tag="dsb")
                nc.vector.tensor_scalar_add(dsb[:], den[:], sk22)
                rden = epool.tile([128, PATCH, PATCH], f32, tag="rden")
                rscr = epool.tile([128, PATCH, PATCH], f32, tag="rscr")
                nc.vector.reciprocal_approx_accurate(rden[:], dsb[:], rscr[:])
                o = opool.tile([128, C, PATCH, PATCH], f32, tag="o")
                for c in range(C):
                    nc.vector.tensor_tensor(o[:, c], num[c][:], rden[:], A.mult)
                nc.vector.tensor_scalar(o[:], o[:], 0.0, 1.0, A.max, A.min)
                nc.sync.dma_start(outd[r], o[:])

    nc.finalize()
    return nc


def _get_module():
    if "nc" not in _CACHE:
        _CACHE["nc"] = _build_module()
    return _CACHE["nc"]


def _patchify(core_imgs):
    from numpy.lib.stride_tricks import sliding_window_view

    xp = np.transpose(core_imgs, (0, 3, 1, 2))
    xpad = np.pad(xp, ((0, 0), (0, 0), (2, 2), (2, 2)), mode="reflect")
    win = sliding_window_view(xpad, (HALO, HALO), axis=(2, 3))[:, :, ::PATCH, ::PATCH]
    pat = np.ascontiguousarray(win.transpose(0, 2, 3, 1, 4, 5)).reshape(
        PATCHES_PER_CORE, C, HALO, HALO
    )
    return pat.reshape(ROUNDS, 128, C, HALO, HALO).astype(np.float32)


def _unpatchify(o):
    o = o.reshape(IMGS_PER_CORE, NPS, NPS, C, PATCH, PATCH)
    o = o.transpose(0, 3, 1, 4, 2, 5).reshape(IMGS_PER_CORE, C, H, W)
    return np.ascontiguousarray(o.transpose(0, 2, 3, 1))


def _make_in_maps(images):
    sk = _space_kernel()
    identb = np.eye(128).astype(np.float16)
    identsk = (np.eye(128) * sk[2, 2]).astype(np.float32)
    lnsk_vals = np.zeros(32, dtype=np.float32)
    for ti, (dy, dx) in enumerate(PAIRS):
        lnsk_vals[ti] = np.log(sk[dy, dx])
    lnsk = np.broadcast_to(lnsk_vals, (128, 32)).copy()
    in_maps = []
    for i in range(NCORES):
        in_maps.append(
            {
                "xpat": _patchify(images[i * IMGS_PER_CORE : (i + 1) * IMGS_PER_CORE]),
                "identb": identb,
                "identsk": identsk,
                "lnsk": lnsk,
            }
        )
    return in_maps


def kernel(images):
    from concourse.bass_utils import run_bass_kernel_spmd

    images = np.asarray(images, dtype=np.float32)
    nc = _get_module()
    in_maps = _make_in_maps(images)
    res = run_bass_kernel_spmd(nc, in_maps, core_ids=list(range(NCORES)))
    out = np.empty((B, H, W, C), dtype=np.float32)
    for i in range(NCORES):
        out[i * IMGS_PER_CORE : (i + 1) * IMGS_PER_CORE] = _unpatchify(
            res.results[i]["out"]
        )
    return out



# revision 5
# speedup vs baseline: 1.7922x; 1.7922x over previous
"""Bilateral filter v2: Δ-form accumulation on 8 trn2 cores.

out = clip(x + num'/den), num'(p) = Σ_t w_t(p)·(x(p+d_t) − x(p)), den = Σ w.
Pair symmetry: v' = w·Δ computed once per unordered pair on the union domain;
gather adds v'(p), scatter subtracts v'(p−d) (negated identity stationary).
Center tap contributes 0 to num' and sk22 to den (added in the epilogue), so
no center matmuls.  All per-pixel DVE ops are stock fp16 tensor_tensor on
planar [3,34,34] patch tiles (one 36x36-halo patch per partition).  PE path
selectable: fp16 ±identity matmuls, or fp8e5 DoubleRow (gather+scatter in one
matmul, moving operand = high-byte bitcast view of the fp16 v' tile).
"""

import sys

sys.path.insert(0, "/opt/trn_rl_repo")

import numpy as np

KERNEL_SIZE = 5
SIGMA_S = 1.0
SIGMA_R = 0.04
INV2SR2 = 0.5 / (SIGMA_R * SIGMA_R)

B, H, W, C = 16, 512, 512, 3
NCORES = 8
IMGS_PER_CORE = B // NCORES
PATCH = 32
HALO = 36
NPS = H // PATCH
PATCHES_PER_CORE = IMGS_PER_CORE * NPS * NPS
ROUNDS = PATCHES_PER_CORE // 128

USE_FP8 = False
DBG_NOMM = False  # strip matmuls to isolate DVE/ACT cost
DBG_DEN = True  # emit den matmuls (fp8 path debug)
DBG_C = 3  # num channels to emit (fp8 path debug)
DBG_H = 4  # chunks per channel (fp8 path debug)

_CACHE = {}

PAIRS = [
    (dy, dx)
    for dy in range(KERNEL_SIZE)
    for dx in range(KERNEL_SIZE)
    if (dy < 2) or (dy == 2 and dx < 2)
]


def _space_kernel():
    x = np.arange(KERNEL_SIZE, dtype=np.float32) - (KERNEL_SIZE // 2)
    g = np.exp(-(x * x) / np.float32(2.0 * SIGMA_S * SIGMA_S)).astype(np.float32)
    g = (g / g.sum()).astype(np.float32)
    return np.outer(g, g).astype(np.float32)


def _build_module(repeat=1, use_fp8=None):
    if use_fp8 is None:
        use_fp8 = USE_FP8
    import concourse.bacc as bacc
    import concourse.mybir as mybir
    import concourse.tile as tile
    from concourse.ap import AP

    f32 = mybir.dt.float32
    f16 = mybir.dt.float16
    f8e5 = mybir.dt.float8e5
    A = mybir.AluOpType
    ACT = mybir.ActivationFunctionType
    sk = _space_kernel()
    sk22 = float(sk[2, 2])

    nc = bacc.Bacc("TRN2", target_bir_lowering=False, debug=False)
    xpat = nc.dram_tensor("xpat", [ROUNDS, 128, C, HALO, HALO], f16, kind="ExternalInput")
    lnsk = nc.dram_tensor("lnsk", [128, 32], f32, kind="ExternalInput")
    identb = nc.dram_tensor("identb", [128, 128], f16, kind="ExternalInput")
    nidentb = nc.dram_tensor("nidentb", [128, 128], f16, kind="ExternalInput")
    id8pm = nc.dram_tensor("id8pm", [128, 2, 128], f8e5, kind="ExternalInput")
    id8pp = nc.dram_tensor("id8pp", [128, 2, 128], f8e5, kind="ExternalInput")
    outd = nc.dram_tensor(
        "out", [repeat * ROUNDS, 128, C, PATCH, PATCH], f16, kind="ExternalOutput"
    )

    def rng_ax(d):
        # union of gather [2,34) and scatter [2-d,34-d) index ranges
        if d >= 0:
            return 2 - d, 34
        return 2, 34 - d

    with tile.TileContext(nc) as tc:
        with (
            tc.tile_pool(name="const", bufs=1) as cpool,
            tc.tile_pool(name="xin", bufs=2) as xpool,
            tc.tile_pool(name="work", bufs=2) as wpool,
            tc.tile_pool(name="outp", bufs=2) as opool,
            tc.tile_pool(name="epi", bufs=2) as epool,
            tc.tile_pool(name="psum", bufs=1, space="PSUM") as ppool,
        ):
            lnsk_t = cpool.tile([128, 32], f32, tag="lnsk")
            nc.sync.dma_start(lnsk_t[:], lnsk[:])
            if use_fp8:
                id8pm_t = cpool.tile([128, 2, 128], f8e5, tag="id8pm")
                nc.sync.dma_start(id8pm_t[:], id8pm[:])
                id8pp_t = cpool.tile([128, 2, 128], f8e5, tag="id8pp")
                nc.sync.dma_start(id8pp_t[:], id8pp[:])
            else:
                identb_t = cpool.tile([128, 128], f16, tag="identb")
                nc.sync.dma_start(identb_t[:], identb[:])
                nidentb_t = cpool.tile([128, 128], f16, tag="nidentb")
                nc.sync.dma_start(nidentb_t[:], nidentb[:])

            for ri, r in [
                (rep * ROUNDS + rr, rr) for rep in range(repeat) for rr in range(ROUNDS)
            ]:
                xt = xpool.tile([128, C, HALO, HALO], f16, tag="xt")
                nc.sync.dma_start(xt[:], xpat[r])

                num = [
                    ppool.tile([128, PATCH, PATCH], f32, tag=f"num{c}", name=f"num{c}")
                    for c in range(C)
                ]
                den = ppool.tile([128, PATCH, PATCH], f32, tag="den")

                for ti, (dy, dx) in enumerate(PAIRS):
                    d_y, d_x = dy - 2, dx - 2
                    u0y, u1y = rng_ax(d_y)
                    u0x, u1x = rng_ax(d_x)
                    sy, sx = u1y - u0y, u1x - u0x

                    dlt = wpool.tile([128, C, 34, 34], f16, tag="dlt")
                    nc.vector.tensor_tensor(
                        dlt[:, :, :sy, :sx],
                        xt[:, :, u0y + d_y : u1y + d_y, u0x + d_x : u1x + d_x],
                        xt[:, :, u0y:u1y, u0x:u1x],
                        A.subtract,
                    )
                    q = wpool.tile([128, C, 34, 34], f16, tag="q")
                    nc.vector.tensor_tensor(
                        q[:, :, :sy, :sx], dlt[:, :, :sy, :sx], dlt[:, :, :sy, :sx],
                        A.mult,
                    )
                    d2 = wpool.tile([128, 34, 34], f16, tag="d2")
                    nc.vector.tensor_tensor(
                        d2[:, :sy, :sx], q[:, 0, :sy, :sx], q[:, 1, :sy, :sx], A.add
                    )
                    nc.vector.tensor_tensor(
                        d2[:, :sy, :sx], d2[:, :sy, :sx], q[:, 2, :sy, :sx], A.add
                    )
                    w = wpool.tile([128, 34, 34], f16, tag="w")
                    nc.scalar.activation(
                        w[:, :sy, :sx],
                        d2[:, :sy, :sx],
                        ACT.Exp,
                        bias=lnsk_t[:, ti : ti + 1],
                        scale=-float(INV2SR2),
                    )
                    vt = wpool.tile([128, C, 34, 34], f16, tag="vt")
                    for c in range(C):
                        nc.vector.tensor_tensor(
                            vt[:, c, :sy, :sx], dlt[:, c, :sy, :sx], w[:, :sy, :sx],
                            A.mult,
                        )

                    gy, gx = 2 - u0y, 2 - u0x  # gather origin in union tile
                    zy, zx = 2 - d_y - u0y, 2 - d_x - u0x  # scatter origin
                    first, last = ti == 0, ti == len(PAIRS) - 1

                    if DBG_NOMM:
                        if first:
                            for c in range(C):
                                vg0 = vt[:, c, gy : gy + 32, gx : gx + 32]
                                for hh in range(2):
                                    nc.tensor.matmul(
                                        num[c][:, 16 * hh : 16 * hh + 16],
                                        identb_t[:],
                                        vg0[:, 16 * hh : 16 * hh + 16],
                                        start=True, stop=True,
                                    )
                            wg0 = w[:, gy : gy + 32, gx : gx + 32]
                            for hh in range(2):
                                nc.tensor.matmul(
                                    den[:, 16 * hh : 16 * hh + 16],
                                    identb_t[:],
                                    wg0[:, 16 * hh : 16 * hh + 16],
                                    start=True, stop=True,
                                )
                        continue
                    if use_fp8:
                        dgs = 68 * (zy - gy) + 2 * (zx - gx)
                        v8 = vt[:].bitcast(f8e5)
                        w8 = w[:].bitcast(f8e5)
                        for c in range(DBG_C):
                            for h in range(DBG_H):
                                off = 2 * (1156 * c) + 68 * (gy + 8 * h) + 2 * gx + 1
                                mv = AP(v8.tensor, off, [[6936, 128], [dgs, 2], [68, 8], [2, 32]])
                                nc.tensor.matmul(
                                    num[c][:, 8 * h : 8 * h + 8],
                                    id8pm_t[:],
                                    mv,
                                    start=first,
                                    stop=last,
                                    perf_mode=mybir.MatmulPerfMode.DoubleRow,
                                )
                        for h in range(4 if DBG_DEN else 0):
                            off = 68 * (gy + 8 * h) + 2 * gx + 1
                            mv = AP(w8.tensor, off, [[2312, 128], [dgs, 2], [68, 8], [2, 32]])
                            nc.tensor.matmul(
                                den[:, 8 * h : 8 * h + 8],
                                id8pp_t[:],
                                mv,
                                start=first,
                                stop=last,
                                perf_mode=mybir.MatmulPerfMode.DoubleRow,
                            )
                    else:
                        for c in range(C):
                            vg = vt[:, c, gy : gy + 32, gx : gx + 32]
                            vs = vt[:, c, zy : zy + 32, zx : zx + 32]
                            for hh in range(2):
                                nc.tensor.matmul(
                                    num[c][:, 16 * hh : 16 * hh + 16],
                                    identb_t[:],
                                    vg[:, 16 * hh : 16 * hh + 16],
                                    start=first,
                                    stop=False,
                                )
                                nc.tensor.matmul(
                                    num[c][:, 16 * hh : 16 * hh + 16],
                                    nidentb_t[:],
                                    vs[:, 16 * hh : 16 * hh + 16],
                                    start=False,
                                    stop=last,
                                )
                        wg = w[:, gy : gy + 32, gx : gx + 32]
                        ws = w[:, zy : zy + 32, zx : zx + 32]
                        for hh in range(2):
                            nc.tensor.matmul(
                                den[:, 16 * hh : 16 * hh + 16],
                                identb_t[:],
                                wg[:, 16 * hh : 16 * hh + 16],
                                start=first,
                                stop=False,
                            )
                            nc.tensor.matmul(
                                den[:, 16 * hh : 16 * hh + 16],
                                identb_t[:],
                                ws[:, 16 * hh : 16 * hh + 16],
                                start=False,
                                stop=last,
                            )

                if use_fp8 and (DBG_C < 3 or not DBG_DEN):
                    # debug: fill unwritten psum regions so epilogue reads are defined
                    for c in range(DBG_C, 3):
                        nc.vector.memset(num[c][:], 0.0)
                    if not DBG_DEN:
                        nc.vector.memset(den[:], 1.0)
                # epilogue: out = clip(x + num'/den', 0, 1), den' = den + sk22
                dsb = epool.tile([128, PATCH, PATCH], f32, tag="dsb")
                nc.vector.tensor_scalar_add(dsb[:], den[:], sk22)
                rden = epool.tile([128, PATCH, PATCH], f32, tag="rden")
                rscr = epool.tile([128, PATCH, PATCH], f32, tag="rscr")
                nc.vector.reciprocal_approx_accurate(rden[:], dsb[:], rscr[:])
                o = opool.tile([128, C, PATCH, PATCH], f16, tag="o")
                for c in range(C):
                    nc.vector.tensor_tensor(o[:, c], num[c][:], rden[:], A.mult)
                nc.vector.tensor_tensor(
                    o[:], o[:], xt[:, :, 2 : 2 + PATCH, 2 : 2 + PATCH], A.add
                )
                nc.vector.tensor_scalar(o[:], o[:], 0.0, 1.0, A.max, A.min)
                nc.sync.dma_start(outd[ri], o[:])

    nc.finalize()
    return nc


def _get_module():
    if "nc" not in _CACHE:
        _CACHE["nc"] = _build_module()
    return _CACHE["nc"]


def _patchify(core_imgs):
    from numpy.lib.stride_tricks import sliding_window_view

    xp = np.transpose(core_imgs, (0, 3, 1, 2))
    xpad = np.pad(xp, ((0, 0), (0, 0), (2, 2), (2, 2)), mode="reflect")
    win = sliding_window_view(xpad, (HALO, HALO), axis=(2, 3))[:, :, ::PATCH, ::PATCH]
    pat = np.ascontiguousarray(win.transpose(0, 2, 3, 1, 4, 5)).reshape(
        PATCHES_PER_CORE, C, HALO, HALO
    )
    return pat.reshape(ROUNDS, 128, C, HALO, HALO).astype(np.float16)


def _unpatchify(o):
    o = o.astype(np.float32).reshape(IMGS_PER_CORE, NPS, NPS, C, PATCH, PATCH)
    o = o.transpose(0, 3, 1, 4, 2, 5).reshape(IMGS_PER_CORE, C, H, W)
    return np.ascontiguousarray(o.transpose(0, 2, 3, 1))


def _make_in_maps(images):
    import concourse.mybir as mybir

    f8np = mybir.dt.np(mybir.dt.float8e5)
    sk = _space_kernel()
    identb = np.eye(128).astype(np.float16)
    nidentb = (-np.eye(128)).astype(np.float16)
    eye = np.eye(128, dtype=np.float32)
    id8pm = np.stack([eye, -eye], axis=1).astype(f8np)
    id8pp = np.stack([eye, eye], axis=1).astype(f8np)
    lnsk_vals = np.zeros(32, dtype=np.float32)
    for ti, (dy, dx) in enumerate(PAIRS):
        lnsk_vals[ti] = np.log(sk[dy, dx])
    lnsk = np.broadcast_to(lnsk_vals, (128, 32)).copy()
    in_maps = []
    for i in range(NCORES):
        in_maps.append(
            {
                "xpat": _patchify(images[i * IMGS_PER_CORE : (i + 1) * IMGS_PER_CORE]),
                "lnsk": lnsk,
                "identb": identb,
                "nidentb": nidentb,
                "id8pm": id8pm,
                "id8pp": id8pp,
            }
        )
    return in_maps


def kernel(images):
    from concourse.bass_utils import run_bass_kernel_spmd

    images = np.asarray(images, dtype=np.float32)
    nc = _get_module()
    in_maps = _make_in_maps(images)
    res = run_bass_kernel_spmd(nc, in_maps, core_ids=list(range(NCORES)))
    out = np.empty((B, H, W, C), dtype=np.float32)
    for i in range(NCORES):
        out[i * IMGS_PER_CORE : (i + 1) * IMGS_PER_CORE] = _unpatchify(
            res.results[i]["out"]
        )
    return out


# revision 6
# speedup vs baseline: 1.9503x; 1.0882x over previous
"""Bilateral filter v2: Δ-form accumulation on 8 trn2 cores.

out = clip(x + num'/den), num'(p) = Σ_t w_t(p)·(x(p+d_t) − x(p)), den = Σ w.
Pair symmetry: v' = w·Δ computed once per unordered pair on the union domain;
gather adds v'(p), scatter subtracts v'(p−d) (negated identity stationary).
Center tap contributes 0 to num' and sk22 to den (added in the epilogue), so
no center matmuls.  All per-pixel DVE ops are stock fp16 tensor_tensor on
planar [3,34,34] patch tiles (one 36x36-halo patch per partition).  PE path
selectable: fp16 ±identity matmuls (default), or fp8e5 DoubleRow (gather+
scatter in one matmul, moving operand = high-byte bitcast view of the fp16 v'
tile).  The fp8 path is kept for reference but crashes current HW: consecutive
DoubleRow matmuls whose moving-AP stride structure differs (the per-pair dgs)
raise NRT_EXEC_UNIT_UNRECOVERABLE; offset-only changes are fine.
"""

import sys

sys.path.insert(0, "/opt/trn_rl_repo")

import numpy as np

KERNEL_SIZE = 5
SIGMA_S = 1.0
SIGMA_R = 0.04
INV2SR2 = 0.5 / (SIGMA_R * SIGMA_R)

B, H, W, C = 16, 512, 512, 3
NCORES = 8
IMGS_PER_CORE = B // NCORES
PATCH = 32
HALO = 36
NPS = H // PATCH
PATCHES_PER_CORE = IMGS_PER_CORE * NPS * NPS
ROUNDS = PATCHES_PER_CORE // 128

USE_FP8 = False

_CACHE = {}

PAIRS = [
    (dy, dx)
    for dy in range(KERNEL_SIZE)
    for dx in range(KERNEL_SIZE)
    if (dy < 2) or (dy == 2 and dx < 2)
]


def _space_kernel():
    x = np.arange(KERNEL_SIZE, dtype=np.float32) - (KERNEL_SIZE // 2)
    g = np.exp(-(x * x) / np.float32(2.0 * SIGMA_S * SIGMA_S)).astype(np.float32)
    g = (g / g.sum()).astype(np.float32)
    return np.outer(g, g).astype(np.float32)


def _build_module(repeat=1, use_fp8=None):
    if use_fp8 is None:
        use_fp8 = USE_FP8
    import concourse.bacc as bacc
    import concourse.mybir as mybir
    import concourse.tile as tile
    from concourse.ap import AP

    f32 = mybir.dt.float32
    f16 = mybir.dt.float16
    f8e5 = mybir.dt.float8e5
    A = mybir.AluOpType
    ACT = mybir.ActivationFunctionType
    sk = _space_kernel()
    sk22 = float(sk[2, 2])

    nc = bacc.Bacc("TRN2", target_bir_lowering=False, debug=False)
    xpat = nc.dram_tensor("xpat", [ROUNDS, 128, C, HALO, HALO], f16, kind="ExternalInput")
    lnsk = nc.dram_tensor("lnsk", [128, 32], f32, kind="ExternalInput")
    identb = nc.dram_tensor("identb", [128, 128], f16, kind="ExternalInput")
    nidentb = nc.dram_tensor("nidentb", [128, 128], f16, kind="ExternalInput")
    id8pm = nc.dram_tensor("id8pm", [128, 2, 128], f8e5, kind="ExternalInput")
    id8pp = nc.dram_tensor("id8pp", [128, 2, 128], f8e5, kind="ExternalInput")
    outd = nc.dram_tensor(
        "out", [repeat * ROUNDS, 128, C, PATCH, PATCH], f16, kind="ExternalOutput"
    )

    def rng_ax(d):
        # union of gather [2,34) and scatter [2-d,34-d) index ranges
        if d >= 0:
            return 2 - d, 34
        return 2, 34 - d

    with tile.TileContext(nc) as tc:
        with (
            tc.tile_pool(name="const", bufs=1) as cpool,
            tc.tile_pool(name="xin", bufs=2) as xpool,
            tc.tile_pool(name="work", bufs=2) as wpool,
            tc.tile_pool(name="outp", bufs=2) as opool,
            tc.tile_pool(name="epi", bufs=2) as epool,
            tc.tile_pool(name="psum", bufs=1, space="PSUM") as ppool,
        ):
            lnsk_t = cpool.tile([128, 32], f32, tag="lnsk")
            nc.sync.dma_start(lnsk_t[:], lnsk[:])
            if use_fp8:
                id8pm_t = cpool.tile([128, 2, 128], f8e5, tag="id8pm")
                nc.sync.dma_start(id8pm_t[:], id8pm[:])
                id8pp_t = cpool.tile([128, 2, 128], f8e5, tag="id8pp")
                nc.sync.dma_start(id8pp_t[:], id8pp[:])
            else:
                identb_t = cpool.tile([128, 128], f16, tag="identb")
                nc.sync.dma_start(identb_t[:], identb[:])
                nidentb_t = cpool.tile([128, 128], f16, tag="nidentb")
                nc.sync.dma_start(nidentb_t[:], nidentb[:])

            for ri, r in [
                (rep * ROUNDS + rr, rr) for rep in range(repeat) for rr in range(ROUNDS)
            ]:
                xt = xpool.tile([128, C, HALO, HALO], f16, tag="xt")
                nc.sync.dma_start(xt[:], xpat[r])

                num = [
                    ppool.tile([128, PATCH, PATCH], f32, tag=f"num{c}", name=f"num{c}")
                    for c in range(C)
                ]
                den = ppool.tile([128, PATCH, PATCH], f32, tag="den")

                for ti, (dy, dx) in enumerate(PAIRS):
                    d_y, d_x = dy - 2, dx - 2
                    u0y, u1y = rng_ax(d_y)
                    u0x, u1x = rng_ax(d_x)
                    sy, sx = u1y - u0y, u1x - u0x

                    dlt = wpool.tile([128, C, 34, 34], f16, tag="dlt")
                    nc.vector.tensor_tensor(
                        dlt[:, :, :sy, :sx],
                        xt[:, :, u0y + d_y : u1y + d_y, u0x + d_x : u1x + d_x],
                        xt[:, :, u0y:u1y, u0x:u1x],
                        A.subtract,
                    )
                    q = wpool.tile([128, C, 34, 34], f16, tag="q")
                    nc.vector.tensor_tensor(
                        q[:, :, :sy, :sx], dlt[:, :, :sy, :sx], dlt[:, :, :sy, :sx],
                        A.mult,
                    )
                    d2 = wpool.tile([128, 34, 34], f16, tag="d2")
                    nc.vector.tensor_tensor(
                        d2[:, :sy, :sx], q[:, 0, :sy, :sx], q[:, 1, :sy, :sx], A.add
                    )
                    nc.vector.tensor_tensor(
                        d2[:, :sy, :sx], d2[:, :sy, :sx], q[:, 2, :sy, :sx], A.add
                    )
                    w = wpool.tile([128, 34, 34], f16, tag="w")
                    nc.scalar.activation(
                        w[:, :sy, :sx],
                        d2[:, :sy, :sx],
                        ACT.Exp,
                        bias=lnsk_t[:, ti : ti + 1],
                        scale=-float(INV2SR2),
                    )
                    vt = wpool.tile([128, C, 34, 34], f16, tag="vt")
                    for c in range(C):
                        nc.vector.tensor_tensor(
                            vt[:, c, :sy, :sx], dlt[:, c, :sy, :sx], w[:, :sy, :sx],
                            A.mult,
                        )

                    gy, gx = 2 - u0y, 2 - u0x  # gather origin in union tile
                    zy, zx = 2 - d_y - u0y, 2 - d_x - u0x  # scatter origin
                    first, last = ti == 0, ti == len(PAIRS) - 1

                    if use_fp8:
                        dgs = 68 * (zy - gy) + 2 * (zx - gx)
                        v8 = vt[:].bitcast(f8e5)
                        w8 = w[:].bitcast(f8e5)
                        for c in range(C):
                            for h in range(4):
                                off = 2 * (1156 * c) + 68 * (gy + 8 * h) + 2 * gx + 1
                                mv = AP(v8.tensor, off, [[6936, 128], [dgs, 2], [68, 8], [2, 32]])
                                nc.tensor.matmul(
                                    num[c][:, 8 * h : 8 * h + 8],
                                    id8pm_t[:],
                                    mv,
                                    start=first,
                                    stop=last,
                                    perf_mode=mybir.MatmulPerfMode.DoubleRow,
                                )
                        for h in range(4):
                            off = 68 * (gy + 8 * h) + 2 * gx + 1
                            mv = AP(w8.tensor, off, [[2312, 128], [dgs, 2], [68, 8], [2, 32]])
                            nc.tensor.matmul(
                                den[:, 8 * h : 8 * h + 8],
                                id8pp_t[:],
                                mv,
                                start=first,
                                stop=last,
                                perf_mode=mybir.MatmulPerfMode.DoubleRow,
                            )
                    else:
                        for c in range(C):
                            vg = vt[:, c, gy : gy + 32, gx : gx + 32]
                            vs = vt[:, c, zy : zy + 32, zx : zx + 32]
                            for hh in range(2):
                                nc.tensor.matmul(
                                    num[c][:, 16 * hh : 16 * hh + 16],
                                    identb_t[:],
                                    vg[:, 16 * hh : 16 * hh + 16],
                                    start=first,
                                    stop=False,
                                )
                                nc.tensor.matmul(
                                    num[c][:, 16 * hh : 16 * hh + 16],
                                    nidentb_t[:],
                                    vs[:, 16 * hh : 16 * hh + 16],
                                    start=False,
                                    stop=last,
                                )
                        wg = w[:, gy : gy + 32, gx : gx + 32]
                        ws = w[:, zy : zy + 32, zx : zx + 32]
                        for hh in range(2):
                            nc.tensor.matmul(
                                den[:, 16 * hh : 16 * hh + 16],
                                identb_t[:],
                                wg[:, 16 * hh : 16 * hh + 16],
                                start=first,
                                stop=False,
                            )
                            nc.tensor.matmul(
                                den[:, 16 * hh : 16 * hh + 16],
                                identb_t[:],
                                ws[:, 16 * hh : 16 * hh + 16],
                                start=False,
                                stop=last,
                            )

                # epilogue: out = clip(x + num'/den', 0, 1), den' = den + sk22
                dsb = epool.tile([128, PATCH, PATCH], f32, tag="dsb")
                nc.vector.tensor_scalar_add(dsb[:], den[:], sk22)
                rden = epool.tile([128, PATCH, PATCH], f32, tag="rden")
                rscr = epool.tile([128, PATCH, PATCH], f32, tag="rscr")
                nc.vector.reciprocal_approx_accurate(rden[:], dsb[:], rscr[:])
                o = opool.tile([128, C, PATCH, PATCH], f16, tag="o")
                for c in range(C):
                    nc.vector.tensor_tensor(o[:, c], num[c][:], rden[:], A.mult)
                nc.vector.tensor_tensor(
                    o[:], o[:], xt[:, :, 2 : 2 + PATCH, 2 : 2 + PATCH], A.add
                )
                nc.vector.tensor_scalar(o[:], o[:], 0.0, 1.0, A.max, A.min)
                nc.sync.dma_start(outd[ri], o[:])

    nc.finalize()
    return nc


def _get_module():
    if "nc" not in _CACHE:
        _CACHE["nc"] = _build_module()
    return _CACHE["nc"]


def _patchify(core_imgs):
    from numpy.lib.stride_tricks import sliding_window_view

    xp = np.transpose(core_imgs, (0, 3, 1, 2))
    xpad = np.pad(xp, ((0, 0), (0, 0), (2, 2), (2, 2)), mode="reflect")
    win = sliding_window_view(xpad, (HALO, HALO), axis=(2, 3))[:, :, ::PATCH, ::PATCH]
    pat = np.ascontiguousarray(win.transpose(0, 2, 3, 1, 4, 5)).reshape(
        PATCHES_PER_CORE, C, HALO, HALO
    )
    return pat.reshape(ROUNDS, 128, C, HALO, HALO).astype(np.float16)


def _unpatchify(o):
    o = o.astype(np.float32).reshape(IMGS_PER_CORE, NPS, NPS, C, PATCH, PATCH)
    o = o.transpose(0, 3, 1, 4, 2, 5).reshape(IMGS_PER_CORE, C, H, W)
    return np.ascontiguousarray(o.transpose(0, 2, 3, 1))


def _make_in_maps(images):
    import concourse.mybir as mybir

    f8np = mybir.dt.np(mybir.dt.float8e5)
    sk = _space_kernel()
    identb = np.eye(128).astype(np.float16)
    nidentb = (-np.eye(128)).astype(np.float16)
    eye = np.eye(128, dtype=np.float32)
    id8pm = np.stack([eye, -eye], axis=1).astype(f8np)
    id8pp = np.stack([eye, eye], axis=1).astype(f8np)
    lnsk_vals = np.zeros(32, dtype=np.float32)
    for ti, (dy, dx) in enumerate(PAIRS):
        lnsk_vals[ti] = np.log(sk[dy, dx])
    lnsk = np.broadcast_to(lnsk_vals, (128, 32)).copy()
    in_maps = []
    for i in range(NCORES):
        in_maps.append(
            {
                "xpat": _patchify(images[i * IMGS_PER_CORE : (i + 1) * IMGS_PER_CORE]),
                "lnsk": lnsk,
                "identb": identb,
                "nidentb": nidentb,
                "id8pm": id8pm,
                "id8pp": id8pp,
            }
        )
    return in_maps


def kernel(images):
    from concourse.bass_utils import run_bass_kernel_spmd

    images = np.asarray(images, dtype=np.float32)
    nc = _get_module()
    in_maps = _make_in_maps(images)
    res = run_bass_kernel_spmd(nc, in_maps, core_ids=list(range(NCORES)))
    out = np.empty((B, H, W, C), dtype=np.float32)
    for i in range(NCORES):
        out[i * IMGS_PER_CORE : (i + 1) * IMGS_PER_CORE] = _unpatchify(
            res.results[i]["out"]
        )
    return out


# revision 7
# speedup vs baseline: 2.0362x; 1.0440x over previous
"""Bilateral filter v4: Δ-form + pair-duo fusion on 8 trn2 cores.

Same math as v2 (out = clip(x + num'/den)), two structural changes:
1. Every pair computes on a full 34x34 window (the 36x36 halo makes this
   always in-bounds), so all pairs are shape-uniform and their elementwise
   stages fuse across PAIR DUOS: one [2,3,34,34] tile per duo, with the
   square as a single 6936-elem op, channel-sum adds and exp as [2,34,34]
   ops, and the weighted product as three [2,34,34] ops.  8 DVE ops per two
   pairs instead of 14 — per-instruction overhead (~0.3us) halves.
2. The spatial weight sk_t moves out of the exp bias into the matmul
   stationary (+-sk_t * I), enabling the bias-free duo-fused exp.
"""

import sys

sys.path.insert(0, "/opt/trn_rl_repo")

import numpy as np

KERNEL_SIZE = 5
SIGMA_S = 1.0
SIGMA_R = 0.04
INV2SR2 = 0.5 / (SIGMA_R * SIGMA_R)

B, H, W, C = 16, 512, 512, 3
NCORES = 8
IMGS_PER_CORE = B // NCORES
PATCH = 32
HALO = 36
NPS = H // PATCH
PATCHES_PER_CORE = IMGS_PER_CORE * NPS * NPS
ROUNDS = PATCHES_PER_CORE // 128

_CACHE = {}

PAIRS = [
    (dy, dx)
    for dy in range(KERNEL_SIZE)
    for dx in range(KERNEL_SIZE)
    if (dy < 2) or (dy == 2 and dx < 2)
]
NDUO = len(PAIRS) // 2


def _space_kernel():
    x = np.arange(KERNEL_SIZE, dtype=np.float32) - (KERNEL_SIZE // 2)
    g = np.exp(-(x * x) / np.float32(2.0 * SIGMA_S * SIGMA_S)).astype(np.float32)
    g = (g / g.sum()).astype(np.float32)
    return np.outer(g, g).astype(np.float32)


def _build_module(repeat=1):
    import concourse.bacc as bacc
    import concourse.mybir as mybir
    import concourse.tile as tile

    f32 = mybir.dt.float32
    f16 = mybir.dt.float16
    A = mybir.AluOpType
    ACT = mybir.ActivationFunctionType
    sk = _space_kernel()
    sk22 = float(sk[2, 2])

    nc = bacc.Bacc("TRN2", target_bir_lowering=False, debug=False)
    xpat = nc.dram_tensor("xpat", [ROUNDS, 128, C, HALO, HALO], f16, kind="ExternalInput")
    statd = nc.dram_tensor("statd", [128, 2 * len(PAIRS), 128], f16, kind="ExternalInput")
    outd = nc.dram_tensor(
        "out", [repeat * ROUNDS, 128, C, PATCH, PATCH], f16, kind="ExternalOutput"
    )

    def origin(d):
        # full 34-window start: gather [2,34) and scatter [2-d,34-d) always inside
        return 2 - d if d > 0 else 2

    with tile.TileContext(nc) as tc:
        with (
            tc.tile_pool(name="const", bufs=1) as cpool,
            tc.tile_pool(name="xin", bufs=2) as xpool,
            tc.tile_pool(name="work", bufs=2) as wpool,
            tc.tile_pool(name="outp", bufs=2) as opool,
            tc.tile_pool(name="epi", bufs=2) as epool,
            tc.tile_pool(name="psum", bufs=1, space="PSUM") as ppool,
        ):
            stat_t = cpool.tile([128, 2 * len(PAIRS), 128], f16, tag="stat")
            nc.sync.dma_start(stat_t[:], statd[:])

            for ri, r in [
                (rep * ROUNDS + rr, rr) for rep in range(repeat) for rr in range(ROUNDS)
            ]:
                xt = xpool.tile([128, C, HALO, HALO], f16, tag="xt")
                nc.sync.dma_start(xt[:], xpat[r])

                num = [
                    ppool.tile([128, PATCH, PATCH], f32, tag=f"num{c}", name=f"num{c}")
                    for c in range(C)
                ]
                den = ppool.tile([128, PATCH, PATCH], f32, tag="den")

                for duo in range(NDUO):
                    tis = (2 * duo, 2 * duo + 1)
                    dlt2 = wpool.tile([128, 2, C, 34, 34], f16, tag="dlt2")
                    for s, ti in enumerate(tis):
                        dy, dx = PAIRS[ti]
                        d_y, d_x = dy - 2, dx - 2
                        u0y, u0x = origin(d_y), origin(d_x)
                        nc.vector.tensor_tensor(
                            dlt2[:, s],
                            xt[:, :, u0y + d_y : u0y + d_y + 34, u0x + d_x : u0x + d_x + 34],
                            xt[:, :, u0y : u0y + 34, u0x : u0x + 34],
                            A.subtract,
                        )
                    q2 = wpool.tile([128, 2, C, 34, 34], f16, tag="q2")
                    nc.vector.tensor_tensor(q2[:], dlt2[:], dlt2[:], A.mult)
                    d2 = wpool.tile([128, 2, 34, 34], f16, tag="d2")
                    nc.vector.tensor_tensor(d2[:], q2[:, :, 0], q2[:, :, 1], A.add)
                    nc.vector.tensor_tensor(d2[:], d2[:], q2[:, :, 2], A.add)
                    w2 = wpool.tile([128, 2, 34, 34], f16, tag="w2")
                    nc.scalar.activation(w2[:], d2[:], ACT.Exp, scale=-float(INV2SR2))
                    vt2 = wpool.tile([128, 2, C, 34, 34], f16, tag="vt2")
                    for c in range(C):
                        nc.vector.tensor_tensor(
                            vt2[:, :, c], dlt2[:, :, c], w2[:], A.mult
                        )

                    for s, ti in enumerate(tis):
                        dy, dx = PAIRS[ti]
                        d_y, d_x = dy - 2, dx - 2
                        u0y, u0x = origin(d_y), origin(d_x)
                        gy, gx = 2 - u0y, 2 - u0x
                        zy, zx = 2 - d_y - u0y, 2 - d_x - u0x
                        first = ti == 0
                        last = ti == len(PAIRS) - 1
                        spos = stat_t[:, 2 * ti]
                        sneg = stat_t[:, 2 * ti + 1]
                        for c in range(C):
                            vg = vt2[:, s, c, gy : gy + 32, gx : gx + 32]
                            vs = vt2[:, s, c, zy : zy + 32, zx : zx + 32]
                            for hh in range(2):
                                nc.tensor.matmul(
                                    num[c][:, 16 * hh : 16 * hh + 16],
                                    spos,
                                    vg[:, 16 * hh : 16 * hh + 16],
                                    start=first,
                                    stop=False,
                                )
                                nc.tensor.matmul(
                                    num[c][:, 16 * hh : 16 * hh + 16],
                                    sneg,
                                    vs[:, 16 * hh : 16 * hh + 16],
                                    start=False,
                                    stop=last,
                                )
                        wg = w2[:, s, gy : gy + 32, gx : gx + 32]
                        ws = w2[:, s, zy : zy + 32, zx : zx + 32]
                        for hh in range(2):
                            nc.tensor.matmul(
                                den[:, 16 * hh : 16 * hh + 16],
                                spos,
                                wg[:, 16 * hh : 16 * hh + 16],
                                start=first,
                                stop=False,
                            )
                            nc.tensor.matmul(
                                den[:, 16 * hh : 16 * hh + 16],
                                spos,
                                ws[:, 16 * hh : 16 * hh + 16],
                                start=False,
                                stop=last,
                            )

                # epilogue: out = clip(x + num'/den', 0, 1), den' = den + sk22
                dsb = epool.tile([128, PATCH, PATCH], f32, tag="dsb")
                nc.vector.tensor_scalar_add(dsb[:], den[:], sk22)
                rden = epool.tile([128, PATCH, PATCH], f32, tag="rden")
                rscr = epool.tile([128, PATCH, PATCH], f32, tag="rscr")
                nc.vector.reciprocal_approx_accurate(rden[:], dsb[:], rscr[:])
                o = opool.tile([128, C, PATCH, PATCH], f16, tag="o")
                for c in range(C):
                    nc.vector.tensor_tensor(o[:, c], num[c][:], rden[:], A.mult)
                nc.vector.tensor_tensor(
                    o[:], o[:], xt[:, :, 2 : 2 + PATCH, 2 : 2 + PATCH], A.add
                )
                nc.vector.tensor_scalar(o[:], o[:], 0.0, 1.0, A.max, A.min)
                nc.sync.dma_start(outd[ri], o[:])

    nc.finalize()
    return nc


def _get_module():
    if "nc" not in _CACHE:
        _CACHE["nc"] = _build_module()
    return _CACHE["nc"]


def _patchify(core_imgs):
    from numpy.lib.stride_tricks import sliding_window_view

    xp = np.transpose(core_imgs, (0, 3, 1, 2))
    xpad = np.pad(xp, ((0, 0), (0, 0), (2, 2), (2, 2)), mode="reflect")
    win = sliding_window_view(xpad, (HALO, HALO), axis=(2, 3))[:, :, ::PATCH, ::PATCH]
    pat = np.ascontiguousarray(win.transpose(0, 2, 3, 1, 4, 5)).reshape(
        PATCHES_PER_CORE, C, HALO, HALO
    )
    return pat.reshape(ROUNDS, 128, C, HALO, HALO).astype(np.float16)


def _unpatchify(o):
    o = o.astype(np.float32).reshape(IMGS_PER_CORE, NPS, NPS, C, PATCH, PATCH)
    o = o.transpose(0, 3, 1, 4, 2, 5).reshape(IMGS_PER_CORE, C, H, W)
    return np.ascontiguousarray(o.transpose(0, 2, 3, 1))


def _make_in_maps(images):
    sk = _space_kernel()
    eye = np.eye(128, dtype=np.float32)
    stat = np.zeros((128, 2 * len(PAIRS), 128), dtype=np.float32)
    for ti, (dy, dx) in enumerate(PAIRS):
        stat[:, 2 * ti] = sk[dy, dx] * eye
        stat[:, 2 * ti + 1] = -sk[dy, dx] * eye
    stat = stat.astype(np.float16)
    in_maps = []
    for i in range(NCORES):
        in_maps.append(
            {
                "xpat": _patchify(images[i * IMGS_PER_CORE : (i + 1) * IMGS_PER_CORE]),
                "statd": stat,
            }
        )
    return in_maps


def kernel(images):
    from concourse.bass_utils import run_bass_kernel_spmd

    images = np.asarray(images, dtype=np.float32)
    nc = _get_module()
    in_maps = _make_in_maps(images)
    res = run_bass_kernel_spmd(nc, in_maps, core_ids=list(range(NCORES)))
    out = np.empty((B, H, W, C), dtype=np.float32)
    for i in range(NCORES):
        out[i * IMGS_PER_CORE : (i + 1) * IMGS_PER_CORE] = _unpatchify(
            res.results[i]["out"]
        )
    return out


# revision 8
# speedup vs baseline: 2.2879x; 1.1236x over previous
"""Bilateral filter v5: Δ-form + pair-QUAD fusion on 8 trn2 cores.

Same math as v2 (out = clip(x + num'/den)), two structural changes:
1. Every pair computes on a full 34x34 window (the 36x36 halo makes this
   always in-bounds), so all pairs are shape-uniform and their elementwise
   stages fuse across PAIR DUOS: one [2,3,34,34] tile per duo, with the
   square as a single 6936-elem op, channel-sum adds and exp as [2,34,34]
   ops, and the weighted product as three [2,34,34] ops.  8 DVE ops per two
   pairs instead of 14 — per-instruction overhead (~0.3us) halves.
2. The spatial weight sk_t moves out of the exp bias into the matmul
   stationary (+-sk_t * I), enabling the bias-free duo-fused exp.
"""

import sys

sys.path.insert(0, "/opt/trn_rl_repo")

import numpy as np

KERNEL_SIZE = 5
SIGMA_S = 1.0
SIGMA_R = 0.04
INV2SR2 = 0.5 / (SIGMA_R * SIGMA_R)

B, H, W, C = 16, 512, 512, 3
NCORES = 8
IMGS_PER_CORE = B // NCORES
PATCH = 32
HALO = 36
NPS = H // PATCH
PATCHES_PER_CORE = IMGS_PER_CORE * NPS * NPS
ROUNDS = PATCHES_PER_CORE // 128

_CACHE = {}

PAIRS = [
    (dy, dx)
    for dy in range(KERNEL_SIZE)
    for dx in range(KERNEL_SIZE)
    if (dy < 2) or (dy == 2 and dx < 2)
]
NQUAD = len(PAIRS) // 4


def _space_kernel():
    x = np.arange(KERNEL_SIZE, dtype=np.float32) - (KERNEL_SIZE // 2)
    g = np.exp(-(x * x) / np.float32(2.0 * SIGMA_S * SIGMA_S)).astype(np.float32)
    g = (g / g.sum()).astype(np.float32)
    return np.outer(g, g).astype(np.float32)


def _build_module(repeat=1):
    import concourse.bacc as bacc
    import concourse.mybir as mybir
    import concourse.tile as tile

    f32 = mybir.dt.float32
    f16 = mybir.dt.float16
    A = mybir.AluOpType
    ACT = mybir.ActivationFunctionType
    sk = _space_kernel()
    sk22 = float(sk[2, 2])

    nc = bacc.Bacc("TRN2", target_bir_lowering=False, debug=False)
    xpat = nc.dram_tensor("xpat", [ROUNDS, 128, C, HALO, HALO], f16, kind="ExternalInput")
    statd = nc.dram_tensor("statd", [128, 2 * len(PAIRS), 128], f16, kind="ExternalInput")
    outd = nc.dram_tensor(
        "out", [repeat * ROUNDS, 128, C, PATCH, PATCH], f16, kind="ExternalOutput"
    )

    def origin(d):
        # full 34-window start: gather [2,34) and scatter [2-d,34-d) always inside
        return 2 - d if d > 0 else 2

    with tile.TileContext(nc) as tc:
        with (
            tc.tile_pool(name="const", bufs=1) as cpool,
            tc.tile_pool(name="xin", bufs=2) as xpool,
            tc.tile_pool(name="work", bufs=2) as wpool,
            tc.tile_pool(name="outp", bufs=2) as opool,
            tc.tile_pool(name="epi", bufs=2) as epool,
            tc.tile_pool(name="psum", bufs=1, space="PSUM") as ppool,
        ):
            stat_t = cpool.tile([128, 2 * len(PAIRS), 128], f16, tag="stat")
            nc.sync.dma_start(stat_t[:], statd[:])

            for ri, r in [
                (rep * ROUNDS + rr, rr) for rep in range(repeat) for rr in range(ROUNDS)
            ]:
                xt = xpool.tile([128, C, HALO, HALO], f16, tag="xt")
                nc.sync.dma_start(xt[:], xpat[r])

                num = [
                    ppool.tile([128, PATCH, PATCH], f32, tag=f"num{c}", name=f"num{c}")
                    for c in range(C)
                ]
                den = ppool.tile([128, PATCH, PATCH], f32, tag="den")

                for quad in range(NQUAD):
                    tis = tuple(4 * quad + j for j in range(4))
                    dlt2 = wpool.tile([128, 4, C, 34, 34], f16, tag="dlt2")
                    for s, ti in enumerate(tis):
                        dy, dx = PAIRS[ti]
                        d_y, d_x = dy - 2, dx - 2
                        u0y, u0x = origin(d_y), origin(d_x)
                        nc.vector.tensor_tensor(
                            dlt2[:, s],
                            xt[:, :, u0y + d_y : u0y + d_y + 34, u0x + d_x : u0x + d_x + 34],
                            xt[:, :, u0y : u0y + 34, u0x : u0x + 34],
                            A.subtract,
                        )
                    # vt2 doubles as the squares scratch: q lives in vt2 until
                    # the channel-sum consumes it, then v' overwrites vt2
                    vt2 = wpool.tile([128, 4, C, 34, 34], f16, tag="vt2")
                    nc.vector.tensor_tensor(vt2[:], dlt2[:], dlt2[:], A.mult)
                    d2 = wpool.tile([128, 4, 34, 34], f16, tag="d2")
                    nc.vector.tensor_tensor(d2[:], vt2[:, :, 0], vt2[:, :, 1], A.add)
                    nc.vector.tensor_tensor(d2[:], d2[:], vt2[:, :, 2], A.add)
                    w2 = wpool.tile([128, 4, 34, 34], f16, tag="w2")
                    nc.scalar.activation(w2[:], d2[:], ACT.Exp, scale=-float(INV2SR2))
                    for c in range(C):
                        nc.vector.tensor_tensor(
                            vt2[:, :, c], dlt2[:, :, c], w2[:], A.mult
                        )

                    for s, ti in enumerate(tis):
                        dy, dx = PAIRS[ti]
                        d_y, d_x = dy - 2, dx - 2
                        u0y, u0x = origin(d_y), origin(d_x)
                        gy, gx = 2 - u0y, 2 - u0x
                        zy, zx = 2 - d_y - u0y, 2 - d_x - u0x
                        first = ti == 0
                        last = ti == len(PAIRS) - 1
                        spos = stat_t[:, 2 * ti]
                        sneg = stat_t[:, 2 * ti + 1]
                        for c in range(C):
                            vg = vt2[:, s, c, gy : gy + 32, gx : gx + 32]
                            vs = vt2[:, s, c, zy : zy + 32, zx : zx + 32]
                            for hh in range(2):
                                nc.tensor.matmul(
                                    num[c][:, 16 * hh : 16 * hh + 16],
                                    spos,
                                    vg[:, 16 * hh : 16 * hh + 16],
                                    start=first,
                                    stop=False,
                                )
                                nc.tensor.matmul(
                                    num[c][:, 16 * hh : 16 * hh + 16],
                                    sneg,
                                    vs[:, 16 * hh : 16 * hh + 16],
                                    start=False,
                                    stop=last,
                                )
                        wg = w2[:, s, gy : gy + 32, gx : gx + 32]
                        ws = w2[:, s, zy : zy + 32, zx : zx + 32]
                        for hh in range(2):
                            nc.tensor.matmul(
                                den[:, 16 * hh : 16 * hh + 16],
                                spos,
                                wg[:, 16 * hh : 16 * hh + 16],
                                start=first,
                                stop=False,
                            )
                            nc.tensor.matmul(
                                den[:, 16 * hh : 16 * hh + 16],
                                spos,
                                ws[:, 16 * hh : 16 * hh + 16],
                                start=False,
                                stop=last,
                            )

                # epilogue: out = clip(x + num'/den', 0, 1), den' = den + sk22
                dsb = epool.tile([128, PATCH, PATCH], f32, tag="dsb")
                nc.vector.tensor_scalar_add(dsb[:], den[:], sk22)
                rden = epool.tile([128, PATCH, PATCH], f32, tag="rden")
                rscr = epool.tile([128, PATCH, PATCH], f32, tag="rscr")
                nc.vector.reciprocal_approx_accurate(rden[:], dsb[:], rscr[:])
                o = opool.tile([128, C, PATCH, PATCH], f16, tag="o")
                for c in range(C):
                    nc.vector.tensor_tensor(o[:, c], num[c][:], rden[:], A.mult)
                nc.vector.tensor_tensor(
                    o[:], o[:], xt[:, :, 2 : 2 + PATCH, 2 : 2 + PATCH], A.add
                )
                nc.vector.tensor_scalar(o[:], o[:], 0.0, 1.0, A.max, A.min)
                nc.sync.dma_start(outd[ri], o[:])

    nc.finalize()
    return nc


def _get_module():
    if "nc" not in _CACHE:
        _CACHE["nc"] = _build_module()
    return _CACHE["nc"]


def _patchify(core_imgs):
    from numpy.lib.stride_tricks import sliding_window_view

    xp = np.transpose(core_imgs, (0, 3, 1, 2))
    xpad = np.pad(xp, ((0, 0), (0, 0), (2, 2), (2, 2)), mode="reflect")
    win = sliding_window_view(xpad, (HALO, HALO), axis=(2, 3))[:, :, ::PATCH, ::PATCH]
    pat = np.ascontiguousarray(win.transpose(0, 2, 3, 1, 4, 5)).reshape(
        PATCHES_PER_CORE, C, HALO, HALO
    )
    return pat.reshape(ROUNDS, 128, C, HALO, HALO).astype(np.float16)


def _unpatchify(o):
    o = o.astype(np.float32).reshape(IMGS_PER_CORE, NPS, NPS, C, PATCH, PATCH)
    o = o.transpose(0, 3, 1, 4, 2, 5).reshape(IMGS_PER_CORE, C, H, W)
    return np.ascontiguousarray(o.transpose(0, 2, 3, 1))


def _make_in_maps(images):
    sk = _space_kernel()
    eye = np.eye(128, dtype=np.float32)
    stat = np.zeros((128, 2 * len(PAIRS), 128), dtype=np.float32)
    for ti, (dy, dx) in enumerate(PAIRS):
        stat[:, 2 * ti] = sk[dy, dx] * eye
        stat[:, 2 * ti + 1] = -sk[dy, dx] * eye
    stat = stat.astype(np.float16)
    in_maps = []
    for i in range(NCORES):
        in_maps.append(
            {
                "xpat": _patchify(images[i * IMGS_PER_CORE : (i + 1) * IMGS_PER_CORE]),
                "statd": stat,
            }
        )
    return in_maps


def kernel(images):
    from concourse.bass_utils import run_bass_kernel_spmd

    images = np.asarray(images, dtype=np.float32)
    nc = _get_module()
    in_maps = _make_in_maps(images)
    res = run_bass_kernel_spmd(nc, in_maps, core_ids=list(range(NCORES)))
    out = np.empty((B, H, W, C), dtype=np.float32)
    for i in range(NCORES):
        out[i * IMGS_PER_CORE : (i + 1) * IMGS_PER_CORE] = _unpatchify(
            res.results[i]["out"]
        )
    return out
